# revision 16
# baseline (speedup 1.0000x reference)
"""Trainium2 Bass kernel for the attention-gate block (sample-major DMA).

Math (per sample n, after folding BN into the convs):
  X     = x[n, :, ::2, ::2].reshape(C, 4)                 # C=512, L=4
  act_k = relu(Wk' @ X + bk')            k=0,1,2          # D=64 each
  S     = act0^T act1  (4x4);  P = softmax_rows(S)
  Z     = P @ act2^T  (4x64)
  Y     = W4' @ Z^T + b4'                                  # (512, 4)
  out[n,c,h,w] = x[n,c,h,w] + Y[c,h]                       # broadcast over w

Device mapping (per core, 256 samples, blocks of 128):
  - SAMPLE-MAJOR DMA: partition = sample, each partition line moves one
    8KB-contiguous chunk of a sample's row -> line-rate HBM.  All bulk
    DMA rides the sync HWDGE ring (loads then stores); the constants go
    out as two packed tensors on the scalar ring.
  - the ::2,::2 gather + channel-major conv layout comes from 16 PE
    transposes per block, 4 per c-chunk into one PSUM bank, evacuated
    by a single bf16 copy per chunk; conv/attention matmuls are bf16.
  - q, k, v are all computed [d, (n l)]-major (v's bias rides the relu
    as a per-partition ACT bias); v is PE-transposed per 32-sample sub
    to feed the P@V matmul.
  - attention per sub: one [64]x[128,128] gram matmul, masked exp
    (ACT+DVE), then P@V and the denominator row share one PSUM tile and
    one evacuation.  Denominators spread onto partitions as [sample, l]
    via 4 rank-1 matmuls per block; one reciprocal per block.
  - GEMM2 keeps the z slice stationary so Y lands sample-major; its 4
    h-outputs fill one 4-bank PSUM tile normalized by a single DVE
    multiply against a broadcast 1/d (bias via the K=65 d-row fold).
  - residual: one tensor_add per chunk (w-broadcast via a step-0 AP)
    into the fp32 x tile in place; engines are strict FIFO so emission
    is phase-major across blocks.
"""

import sys

for _p in ("/opt/trn_rl_repo",):
    if _p not in sys.path:
        sys.path.insert(0, _p)

import numpy as np
import ml_dtypes

import concourse.mybir as mybir
from concourse import bacc, tile

EPS = 1e-5
N_TOTAL, C, D, HH, WW = 2048, 512, 64, 4, 4
NCORES = 8
NSH = N_TOTAL // NCORES  # 256 samples per core
BLK = 128                # samples per block (= partition dim)
SUB = 32                 # samples per attention subchunk
NCH = 4                  # c-chunks of 128 channels
SHIFT = -34.0            # constant exp shift; cancels in the normalization
F32 = mybir.dt.float32
BF16 = mybir.dt.bfloat16

# packed-constant column offsets (bf16 tensor)
_CB_WQ, _CB_WK, _CB_W2 = 0, 256, 512
_CB_W4A, _CB_MSK, _CB_EY4, _CB_IDB = 768, 1280, 1408, 1424
_CB_COLS = 1552
# fp32 tensor: identity | bq | bk | b2
_CF_COLS = 131

_PROG_CACHE = {}


def build_program(nsh=NSH, blk=BLK, reps=1):
    key = (nsh, blk, reps)
    if key in _PROG_CACHE:
        return _PROG_CACHE[key]
    assert blk == 128 and nsh % blk == 0

    nc = bacc.Bacc("TRN2", target_bir_lowering=False, debug=False)
    AF = mybir.ActivationFunctionType

    x_in = nc.dram_tensor("x", (nsh, C, HH, WW), F32, kind="ExternalInput")
    cbf = nc.dram_tensor("cbf", (128, _CB_COLS), BF16, kind="ExternalInput")
    cf32 = nc.dram_tensor("cf32", (128, _CF_COLS), F32, kind="ExternalInput")
    out = nc.dram_tensor("out", (nsh, C, HH, WW), F32, kind="ExternalOutput")

    nblk = nsh // blk
    CH = 2048  # elems per c-chunk of an x row: 128 c * 16 hw

    with tile.TileContext(nc) as tc:
        with (
            tc.tile_pool(name="const", bufs=1) as cpool,
            tc.tile_pool(name="xc", bufs=10) as xpool,
            tc.tile_pool(name="xsT", bufs=9) as tpool,
            tc.tile_pool(name="work", bufs=4) as wpool,
            tc.tile_pool(name="att", bufs=6) as apool,
            tc.tile_pool(name="ynm", bufs=2) as ypool,
            tc.tile_pool(name="ps", bufs=2, space="PSUM") as psA,
            tc.tile_pool(name="pst", bufs=2, space="PSUM") as psT,
            tc.tile_pool(name="psy", bufs=1, space="PSUM") as psY,
        ):
            cbf_sb = cpool.tile([128, _CB_COLS], BF16)
            nc.scalar.dma_start(cbf_sb[:], cbf[:])
            cf_sb = cpool.tile([128, _CF_COLS], F32)
            nc.scalar.dma_start(cf_sb[:], cf32[:])

            wq_sb = cbf_sb[:, _CB_WQ:_CB_WQ + 256].rearrange(
                "p (k d) -> p k d", k=4)
            wk_sb = cbf_sb[:, _CB_WK:_CB_WK + 256].rearrange(
                "p (k d) -> p k d", k=4)
            w2_sb = cbf_sb[:, _CB_W2:_CB_W2 + 256].rearrange(
                "p (k d) -> p k d", k=4)
            w4f = cbf_sb[0:D + 1, _CB_W4A:_CB_W4A + 512]
            msk_sb = cbf_sb[:, _CB_MSK:_CB_MSK + 128]
            ey4_sb = cbf_sb[0:1, _CB_EY4:_CB_EY4 + 16]
            idb_sb = cbf_sb[0:D, _CB_IDB:_CB_IDB + D]
            idn_sb = cf_sb[:, 0:128]
            bq_sb = cf_sb[0:D, 128:129]
            bk_sb = cf_sb[0:D, 129:130]
            b2_sb = cf_sb[0:D, 130:131]
            ones_c = cpool.tile([128, 1], BF16)
            nc.vector.memset(ones_c[:], 1.0)
            shift_sb = cpool.tile([128, 1], F32)
            nc.vector.memset(shift_sb[:], SHIFT)

            # sample-major views: one 8KB-contiguous run per (sample, chunk)
            xv = x_in[:].rearrange("(b n) c h w -> b n (c h w)", n=blk)
            ov = out[:].rearrange("(b n) c h w -> b n (c h w)", n=blk)

            blocks = [b for _ in range(reps) for b in range(nblk)]
            NB = len(blocks)
            st_xc = [None] * NB
            st_xsT = [None] * NB
            st_act = [None] * NB
            st_z = [None] * NB
            st_r = [None] * NB
            st_y = [None] * NB

            for i, b in enumerate(blocks):
                x_c = []
                for k in range(NCH):
                    xt = xpool.tile([128, CH], F32, tag="xc")
                    nc.sync.dma_start(xt[:], xv[b, :, k * CH:(k + 1) * CH])
                    x_c.append(xt)
                st_xc[i] = x_c

            # ---- transpose the ::2,::2 picks to channel-major bf16 ----
            for i in range(NB):
                xsT = []
                for k in range(NCH):
                    xst = tpool.tile([128, 128, 4], BF16, tag="xsT")
                    xcv = st_xc[i][k][:].rearrange(
                        "p (c h w) -> p c h w", h=4, w=4)
                    ps_t = psT.tile([128, 4, 128], F32, tag="pst")
                    for l in range(4):
                        hp, wp = (l // 2) * 2, (l % 2) * 2
                        nc.tensor.transpose(ps_t[:, l], xcv[:, :, hp, wp],
                                            idn_sb)
                    xtv = xst[:].rearrange("p n l -> p l n")
                    if k % 2 == 0:
                        nc.scalar.activation(xtv, ps_t[:], AF.Copy)
                    else:
                        nc.vector.tensor_copy(xtv, ps_t[:])
                    xsT.append(xst)
                st_xsT[i] = xsT

            # ---- GEMM1: q, k, v over 4 c-chunks, all [d, (n l)] ----
            for i in range(NB):
                xsT = st_xsT[i]
                xfs = [xsT[k][:].rearrange("p n l -> p (n l)")
                       for k in range(NCH)]
                ps_q = psA.tile([D, 512], F32, tag="ps")
                for k in range(NCH):
                    nc.tensor.matmul(ps_q[:], lhsT=wq_sb[:, k], rhs=xfs[k],
                                     start=(k == 0), stop=(k == 3))
                a_q = wpool.tile([D, 512], BF16, tag="aq")
                nc.scalar.activation(a_q[:], ps_q[:], AF.Relu, bias=bq_sb)
                ps_k = psA.tile([D, 512], F32, tag="ps")
                for k in range(NCH):
                    nc.tensor.matmul(ps_k[:], lhsT=wk_sb[:, k], rhs=xfs[k],
                                     start=(k == 0), stop=(k == 3))
                a_k = wpool.tile([D, 512], BF16, tag="ak")
                nc.scalar.activation(a_k[:], ps_k[:], AF.Relu, bias=bk_sb)
                ps_v = psA.tile([D, 512], F32, tag="ps")
                for k in range(NCH):
                    nc.tensor.matmul(ps_v[:], lhsT=w2_sb[:, k], rhs=xfs[k],
                                     start=(k == 0), stop=(k == 3))
                a_v = wpool.tile([D, 512], BF16, tag="av")
                nc.scalar.activation(a_v[:], ps_v[:], AF.Relu, bias=b2_sb)
                st_act[i] = (a_q, a_k, a_v)

            # ---- attention per 32-sample sub ----
            for i in range(NB):
                a_q, a_k, a_v = st_act[i]
                z_all = apool.tile([D + 1, 4, 4, SUB], BF16, tag="z")
                for s in range(4):
                    cl = slice(s * 128, (s + 1) * 128)
                    # v transposed for this sub: [128 (n l), 64 d]
                    ps_a2 = psT.tile([128, D], BF16, tag="pst")
                    nc.tensor.transpose(ps_a2[:], a_v[:, cl], idb_sb)
                    a2t = apool.tile([128, D], BF16, tag="a2t")
                    if s % 2 == 0:
                        nc.vector.tensor_copy(a2t[:], ps_a2[:])
                    else:
                        nc.scalar.activation(a2t[:], ps_a2[:], AF.Copy)

                    ps_g = psA.tile([128, 128], F32, tag="ps")
                    nc.tensor.matmul(ps_g[:], lhsT=a_k[:, cl], rhs=a_q[:, cl],
                                     start=True, stop=True)
                    e_t = apool.tile([128, 128], BF16, tag="e")
                    nc.scalar.activation(e_t[:], ps_g[:], AF.Exp,
                                         bias=shift_sb[:])
                    p0 = apool.tile([128, 128], BF16, tag="p0")
                    nc.vector.tensor_mul(p0[:], e_t[:], msk_sb)

                    # z rows 0..63 + denominator row 64 share one PSUM tile
                    ps_zd = psA.tile([D + 1, 128], F32, tag="ps")
                    nc.tensor.matmul(ps_zd[0:D, :], lhsT=a2t[:], rhs=p0[:],
                                     start=True, stop=True)
                    nc.tensor.matmul(ps_zd[D:D + 1, :], lhsT=ones_c[:],
                                     rhs=p0[:], start=True, stop=True,
                                     skip_group_check=True)
                    zdst = z_all[:, :, s, :].rearrange("p l n -> p n l")
                    zsrc = ps_zd[:].rearrange("p (n l) -> p n l", l=4)
                    if s % 2 == 0:
                        nc.scalar.activation(zdst, zsrc, AF.Copy)
                    else:
                        nc.vector.tensor_copy(zdst, zsrc)
                st_z[i] = z_all

            # ---- denominators onto partitions as [sample, l]; 1/d ----
            for i in range(NB):
                z_all = st_z[i]
                r_ps = psT.tile([128, 4], F32, tag="pst")
                d_blk = wpool.tile([1, 4, 128], BF16, tag="dblk")
                nc.scalar.activation(
                    d_blk[:],
                    z_all[D:D + 1].rearrange("p l s n -> p l (s n)"),
                    AF.Copy)
                for l in range(4):
                    nc.tensor.matmul(
                        r_ps[:], lhsT=d_blk[:, l],
                        rhs=ey4_sb[0:1, l * 4:(l + 1) * 4],
                        start=(l == 0), stop=(l == 3),
                    )
                r_nm = wpool.tile([128, 4], F32, tag="r")
                nc.vector.reciprocal(r_nm[:], r_ps[:])
                st_r[i] = r_nm

            # ---- GEMM2: z stationary -> sample-major Y; one 4-bank PSUM
            # tile normalized by a single DVE multiply ----
            for i in range(NB):
                z_all, r_nm = st_z[i], st_r[i]
                ps_y = psY.tile([128, 4, 512], F32, tag="psy")
                for h in range(4):
                    nc.tensor.matmul(
                        ps_y[:, h],
                        lhsT=z_all[:, h].rearrange("p s n -> p (s n)"),
                        rhs=w4f, start=True, stop=True)
                y_all = ypool.tile([128, 512, 4], BF16, tag="y")
                rb = r_nm[:].unsqueeze(2).broadcast_to((128, 4, 512))
                nc.vector.tensor_mul(
                    y_all[:].rearrange("p c h -> p h c"), ps_y[:], rb)
                st_y[i] = y_all

            # ---- residual add + store ----
            for i, b in enumerate(blocks):
                x_c, y_all = st_xc[i], st_y[i]
                for k in range(NCH):
                    xc4 = x_c[k][:].rearrange("p (c h w) -> p c h w",
                                              h=4, w=4)
                    ynb = (y_all[:, k * 128:(k + 1) * 128, :]
                           .unsqueeze(3).broadcast_to((128, 128, 4, 4)))
                    eng = nc.vector if k < 3 else nc.gpsimd
                    eng.tensor_add(xc4[:], ynb, xc4[:])
                    nc.sync.dma_start(ov[b, :, k * CH:(k + 1) * CH],
                                      x_c[k][:])

    nc.compile()
    _PROG_CACHE[key] = nc
    return nc


def prep_params(W123, b123, g123, be123, m123, v123, W4, b4, g4, be4, m4, v4):
    """Fold BN into the convs; pack all constants into two tensors."""
    f32, bf = np.float32, ml_dtypes.bfloat16
    s123 = (g123 / np.sqrt(v123 + EPS)).astype(f32)            # (3, D)
    Wf = (W123 * s123[:, :, None]).astype(f32)                 # (3, D, C)
    bf123 = ((b123 - m123) * s123 + be123).astype(f32)         # (3, D)
    s4 = (g4 / np.sqrt(v4 + EPS)).astype(f32)                  # (C,)
    W4f = (W4 * s4[:, None]).astype(f32)                       # (C, D)
    b4f = ((b4 - m4) * s4 + be4).astype(f32)                   # (C,)

    def chunks(wt):  # (C, D) -> (128, NCH*D)
        return wt.reshape(NCH, 128, D).transpose(1, 0, 2).reshape(128, -1)

    cbf = np.zeros((128, _CB_COLS), f32)
    cbf[:, _CB_WQ:_CB_WQ + 256] = chunks(Wf[0].T)
    cbf[:, _CB_WK:_CB_WK + 256] = chunks(Wf[1].T)
    cbf[:, _CB_W2:_CB_W2 + 256] = chunks(Wf[2].T)
    w4a = np.concatenate([W4f.T, b4f[None, :]], axis=0)        # (65, C)
    cbf[0:D + 1, _CB_W4A:_CB_W4A + 512] = w4a
    cbf[:, _CB_MSK:_CB_MSK + 128] = np.kron(
        np.eye(SUB, dtype=f32), np.ones((4, 4), f32))
    cbf[0:1, _CB_EY4:_CB_EY4 + 16] = np.eye(4, dtype=f32).reshape(1, 16)
    cbf[0:D, _CB_IDB:_CB_IDB + D] = np.eye(D, dtype=f32)

    cf32 = np.zeros((128, _CF_COLS), f32)
    cf32[:, 0:128] = np.eye(128, dtype=f32)
    cf32[0:D, 128] = bf123[0]
    cf32[0:D, 129] = bf123[1]
    cf32[0:D, 130] = bf123[2]
    return dict(cbf=cbf.astype(bf), cf32=cf32)


def _run(inputs, trace=False, **spmd_kwargs):
    from concourse.bass_utils import run_bass_kernel_spmd

    x = np.ascontiguousarray(np.asarray(inputs["x"], dtype=np.float32))
    params = prep_params(**{k: np.asarray(v, np.float64)
                            for k, v in inputs.items() if k != "x"})
    nc = build_program()
    in_maps = [
        {"x": x[i * NSH:(i + 1) * NSH], **params} for i in range(NCORES)
    ]
    res = run_bass_kernel_spmd(
        nc, in_maps, list(range(NCORES)), trace=trace, **spmd_kwargs
    )
    outs = np.concatenate(
        [np.asarray(res.results[i]["out"]) for i in range(NCORES)], axis=0
    )
    return outs, res


def kernel(**inputs):
    outs, _ = _run(inputs)
    return outs
